# revision 4
# baseline (speedup 1.0000x reference)
"""Trainium2 Bass kernel for nn_ChatModel_7292854469026 (self-contained).

4-layer transformer with MoE (top-2 of 4 experts), RoPE attention, tied dims:
B=2 S=1024 H=1024 NH=16 HD=64 L=4 FF=2560 E=4 V=32000 K=2.

Sharding over 8 NeuronCores:
- attention tensor-parallel: 2 heads per core (wq/wk/wv column shard, wo row
  shard), partial outputs AllReduced.
- MoE expert-parallel: core c handles expert c//2, FF half c%2 (dense over all
  tokens, output scaled by the top-2 combine weight, AllReduced).
- lm_head vocab-parallel: 4000 columns per core, host concatenates.

Activations live feature-major (x^T [H, T]) in SBUF; cross-partition sums use
ones-matmuls; per-token scales use K=1 rank-1 broadcast matmuls.
"""
import contextlib
import ctypes
import sys
import types

import numpy as np
import ml_dtypes

import concourse.bass as bass
import concourse.mybir as mybir
import concourse.tile as tile
from concourse import bacc, bass_utils
from concourse.masks import make_identity

F32 = mybir.dt.float32
BF16 = mybir.dt.bfloat16
AF = mybir.ActivationFunctionType
ALU = mybir.AluOpType
AX = mybir.AxisListType

B, S, H, NH, HD, L, FF, E, V, K = 2, 1024, 1024, 16, 64, 4, 2560, 4, 32000, 2
T = B * S
EPS = 1e-5
ROPE_BASE = 10000.0
NC = 8
HL = NH * HD // NC          # 128 local attention dims (2 heads)
FFL = FF // 2               # 1280 FF half per core
VL = V // NC                # 4000 vocab cols per core
KT = H // 128               # 8 k-tiles over H
MT = FFL // 128             # 10 m-tiles over FF half
TT = T // 128               # 16 token tiles
NCH = T // 512              # 4 column chunks of 512
MASKV = -1e4

LAST_EXEC_NS = None

bf = lambda a: np.ascontiguousarray(a).astype(ml_dtypes.bfloat16)
f32 = lambda a: np.ascontiguousarray(a, dtype=np.float32)


def _install_ntff_hook():
    """Best-effort NTFF profile hook (for HW exec timing under axon)."""
    try:
        import antenv
        if "antenv.axon_hooks" in sys.modules:
            return True
        lib = ctypes.CDLL("/opt/axon/libaxon_pjrt.so")
        if not hasattr(lib, "axon_start_nrt_profile"):
            return False
        lib.axon_start_nrt_profile.argtypes = [ctypes.POINTER(ctypes.c_int64),
                                               ctypes.c_size_t]
        lib.axon_start_nrt_profile.restype = ctypes.c_int64
        lib.axon_stop_nrt_profile.argtypes = [ctypes.c_char_p]
        lib.axon_stop_nrt_profile.restype = ctypes.c_int64

        @contextlib.contextmanager
        def _hook(output_dir, device_ids):
            import jax
            jax.devices()
            if device_ids:
                ids = (ctypes.c_int64 * len(device_ids))(*device_ids)
                rc = lib.axon_start_nrt_profile(ids, len(device_ids))
            else:
                rc = lib.axon_start_nrt_profile(None, 0)
            if rc != 0:
                raise RuntimeError(f"axon_start_nrt_profile rc={rc}")
            try:
                yield
            finally:
                n = lib.axon_stop_nrt_profile(str(output_dir).encode())
                print(f"profile: {n} file(s) -> {output_dir}", file=sys.stderr)

        holder = [_hook]
        mod = types.ModuleType("antenv.axon_hooks")
        mod.get_axon_ntff_profile_hook = lambda: holder[0]
        mod.set_axon_ntff_profile_hook = lambda h: holder.__setitem__(0, h)
        sys.modules["antenv.axon_hooks"] = mod
        antenv.axon_hooks = mod
        return True
    except Exception:
        return False


def build(with_biases):
    gb_any, egb_any, eub_any, edb_any = with_biases
    nc = bacc.Bacc("TRN2", target_bir_lowering=False, debug=False,
                   num_devices=NC)
    di = {}

    def din(name, shape, dt):
        di[name] = nc.dram_tensor(name, list(shape), dt, kind="ExternalInput").ap()
        return di[name]

    x0T = din("x0T", [H, T], F32)                     # emb[ids].T * sqrt(H)
    wqkvo = din("wqkvo", [L, 32, 128, 128], BF16)     # q8|k8|v8|o8 lhsT tiles
    gw = din("gw", [L, KT, 128, E], F32)
    gbb = din("gbb", [L, 128, E], F32) if gb_any else None
    egw = din("egw", [L, MT, KT, 128, 128], BF16)
    euw = din("euw", [L, MT, KT, 128, 128], BF16)
    edw = din("edw", [L, KT, MT, 128, 128], BF16)
    egb = din("egb", [L, 128, MT], F32) if egb_any else None
    eub = din("eub", [L, 128, MT], F32) if eub_any else None
    edb = din("edb", [L, 128, KT], F32) if edb_any else None
    wlm = din("wlm", [KT, 128, VL], BF16)
    cosT = din("cosT", [32, S], F32)
    sinT = din("sinT", [32, S], F32)
    mask4 = din("mask4", [128, 4 * 512], F32)
    esel = din("esel", [E, 128], F32)

    logits = nc.dram_tensor("logits", [T, VL], F32, kind="ExternalOutput").ap()
    bal_o = nc.dram_tensor("bal", [1, 1], F32, kind="ExternalOutput").ap()

    with tile.TileContext(nc) as tc:
        with (
            tc.tile_pool(name="cst", bufs=1) as cst,
            tc.tile_pool(name="big", bufs=1) as bigp,
            tc.tile_pool(name="sb", bufs=2) as sb,
            tc.tile_pool(name="per", bufs=1) as per,
            tc.tile_pool(name="wp", bufs=2) as wp,
            tc.tile_pool(name="ps2", bufs=2, space="PSUM") as ps2,
            tc.tile_pool(name="ps1", bufs=1, space="PSUM") as ps1,
            tc.tile_pool(name="dr", bufs=1, space="DRAM") as dr,
        ):
            ident = cst.tile([128, 128], F32, tag="ident")
            make_identity(nc, ident[:])
            onef = cst.tile([128, 128], F32, tag="onef")
            nc.vector.memset(onef[:], 1.0)
            oneb = cst.tile([128, 1], BF16, tag="oneb")
            nc.vector.memset(oneb[:], 1.0)
            cosv = cst.tile([32, S], F32, tag="cosv")
            sinv = cst.tile([32, S], F32, tag="sinv")
            nc.sync.dma_start(cosv[:], cosT[:])
            nc.sync.dma_start(sinv[:], sinT[:])
            maskv = cst.tile([128, 4 * 512], F32, tag="maskv")
            nc.sync.dma_start(maskv[:], mask4[:])
            eselv = cst.tile([E, 128], F32, tag="eselv")
            nc.sync.dma_start(eselv[:], esel[:])
            balsb = cst.tile([1, 1], F32, tag="balsb")
            nc.vector.memset(balsb[:], 0.0)

            # residual x^T as 8 k-tiles in one wide tile [128, KT*T]
            xres = bigp.tile([128, KT * T], F32, tag="xres")
            for k in range(KT):
                nc.sync.dma_start(xres[:, k * T:(k + 1) * T],
                                  x0T[128 * k:128 * (k + 1), :])

            xk = lambda k: xres[:, k * T:(k + 1) * T]

            # AllReduce bounce buffers (internal DRAM)
            arin = dr.tile([H, T], F32, tag="arin")
            arout = dr.tile([H, T], F32, tag="arout")
            arin2 = dr.tile([H, T], F32, tag="arin2")
            arout2 = dr.tile([H, T], F32, tag="arout2")

            def rmsnorm_rstd():
                """Return sbuf row [1, T] of 1/sqrt(mean(x^2)+eps)."""
                rrow = per.tile([1, T], F32, tag="rrow")
                for n in range(NCH):
                    cs = slice(512 * n, 512 * (n + 1))
                    pssq = ps1.tile([1, 512], F32, tag="prow")
                    for k in range(KT):
                        xsq = sb.tile([128, 512], BF16, tag="xsq")
                        nc.vector.tensor_tensor(out=xsq[:], in0=xk(k)[:, cs],
                                                in1=xk(k)[:, cs], op=ALU.mult)
                        nc.tensor.matmul(pssq[:], oneb[:, 0:1], xsq[:],
                                         start=(k == 0), stop=(k == KT - 1))
                    tmp = sb.tile([1, 512], F32, tag="rtmp")
                    nc.vector.tensor_scalar(out=tmp[:], in0=pssq[:],
                                            scalar1=1.0 / H, scalar2=EPS,
                                            op0=ALU.mult, op1=ALU.add)
                    nc.scalar.activation(tmp[:], tmp[:], AF.Sqrt)
                    nc.vector.reciprocal(rrow[:, cs], tmp[:])
                return rrow

            def bcast_row(row, cs):
                """psum [128, 512] = broadcast of row[0:1, cs] down partitions."""
                pbc = ps1.tile([128, 512], F32, tag="pbc")
                nc.tensor.matmul(pbc[:], onef[0:1, :], row[0:1, cs],
                                 start=True, stop=True)
                return pbc

            def norm_chunk(rrow, n, tag):
                """h^T chunk [128, KT, 512] bf16 = x^T * rstd (norm w folded)."""
                cs = slice(512 * n, 512 * (n + 1))
                pbc = bcast_row(rrow, cs)
                hc = per.tile([128, KT * 512], BF16, tag=tag)
                for k in range(KT):
                    nc.vector.tensor_tensor(out=hc[:, 512 * k:512 * (k + 1)],
                                            in0=xk(k)[:, cs], in1=pbc[:],
                                            op=ALU.mult)
                return hc

            def allreduce(bin_, bout):
                for c in range(4):
                    rs = slice(256 * c, 256 * (c + 1))
                    nc.gpsimd.collective_compute(
                        "AllReduce", ALU.add,
                        replica_groups=[list(range(NC))],
                        ins=[bin_[rs, :]], outs=[bout[rs, :]])

            def add_from(bout):
                for k in range(KT):
                    for n in range(NCH):
                        cs = slice(512 * n, 512 * (n + 1))
                        t = sb.tile([128, 512], F32, tag="arld")
                        nc.sync.dma_start(t[:], bout[128 * k:128 * (k + 1), cs])
                        nc.vector.tensor_add(xk(k)[:, cs], xk(k)[:, cs], t[:])

            for l in range(L):
                # ================= attention =================
                rrow = rmsnorm_rstd()
                wt = per.tile([128, 32 * 128], BF16, tag="wqkvo")
                nc.sync.dma_start(
                    wt[:].rearrange("p (a b) -> p a b", a=32),
                    wqkvo[l].rearrange("a p b -> p a b"))
                qT = per.tile([128, T], BF16, tag="qT")
                kT_ = per.tile([128, T], BF16, tag="kT")
                attnT = per.tile([128, T], BF16, tag="attnT")
                vaug = per.tile([128, TT * 130], BF16, tag="vaug")

                for n in range(NCH):
                    hc = norm_chunk(rrow, n, "hc")
                    scs = slice(512 * (n % 2), 512 * (n % 2) + 512)  # rope pos
                    for which, dst in ((0, qT), (1, kT_)):
                        pq = ps2.tile([128, 512], F32, tag="pbig")
                        for k in range(KT):
                            nc.tensor.matmul(
                                pq[:], wt[:, (which * 8 + k) * 128:
                                          (which * 8 + k + 1) * 128],
                                hc[:, 512 * k:512 * (k + 1)],
                                start=(k == 0), stop=(k == KT - 1))
                        # RoPE: rows per head: [x1(32) x2(32)] -> rotated
                        cs = slice(512 * n, 512 * (n + 1))
                        for hh in range(2):
                            r1 = slice(64 * hh, 64 * hh + 32)
                            r2 = slice(64 * hh + 32, 64 * hh + 64)
                            t1 = sb.tile([32, 512], F32, tag="rope")
                            t2 = sb.tile([32, 512], F32, tag="rope")
                            nc.vector.tensor_tensor(out=t1[:], in0=pq[r1, :],
                                                    in1=cosv[:, scs], op=ALU.mult)
                            nc.vector.tensor_tensor(out=t2[:], in0=pq[r2, :],
                                                    in1=sinv[:, scs], op=ALU.mult)
                            nc.vector.tensor_sub(dst[r1, cs], t1[:], t2[:])
                            t3 = sb.tile([32, 512], F32, tag="rope")
                            t4 = sb.tile([32, 512], F32, tag="rope")
                            nc.vector.tensor_tensor(out=t3[:], in0=pq[r1, :],
                                                    in1=sinv[:, scs], op=ALU.mult)
                            nc.vector.tensor_tensor(out=t4[:], in0=pq[r2, :],
                                                    in1=cosv[:, scs], op=ALU.mult)
                            nc.vector.tensor_add(dst[r2, cs], t3[:], t4[:])
                    # v (token-major) for the 4 t-tiles of this chunk
                    for tt_ in range(4):
                        t = 4 * n + tt_
                        pv = ps2.tile([128, 128], F32, tag="pbig")
                        for k in range(KT):
                            nc.tensor.matmul(
                                pv[:], hc[:, 512 * k + 128 * tt_:
                                          512 * k + 128 * (tt_ + 1)],
                                wt[:, (16 + k) * 128:(16 + k + 1) * 128],
                                start=(k == 0), stop=(k == KT - 1))
                        vb = 130 * t
                        nc.vector.tensor_copy(
                            vaug[:, vb:vb + 130]
                            .rearrange("p (g x) -> p g x", g=2)[:, :, 0:64],
                            pv[:].rearrange("p (g x) -> p g x", g=2))
                        nc.vector.memset(vaug[:, vb + 64:vb + 65], 1.0)
                        nc.vector.memset(vaug[:, vb + 129:vb + 130], 1.0)

                # scores -> exp -> attnV per (batch, tq-chunk)
                for b in range(B):
                    for j in range(2):
                        pA = ps1.tile([65, 512], F32, tag="pattA")
                        pB = ps1.tile([65, 512], F32, tag="pattB")
                        ncn = 4 * (j + 1)
                        for c in range(ncn):
                            for hh, patt in ((0, pA), (1, pB)):
                                hs = slice(64 * hh, 64 * (hh + 1))
                                psc = ps2.tile([128, 512], F32, tag="pbig")
                                nc.tensor.matmul(
                                    psc[:],
                                    kT_[hs, 1024 * b + 128 * c:
                                        1024 * b + 128 * (c + 1)],
                                    qT[hs, 1024 * b + 512 * j:
                                       1024 * b + 512 * (j + 1)],
                                    start=True, stop=True)
                                jj = c - 4 * j
                                if jj >= 0:
                                    nc.vector.tensor_tensor(
                                        out=psc[:], in0=psc[:],
                                        in1=maskv[:, 512 * jj:512 * (jj + 1)],
                                        op=ALU.add)
                                et = sb.tile([128, 512], BF16, tag="et")
                                nc.scalar.activation(et[:], psc[:], AF.Exp,
                                                     scale=0.125)
                                vb = 130 * (8 * b + c)
                                nc.tensor.matmul(
                                    patt[:],
                                    vaug[:, vb + 65 * hh:vb + 65 * (hh + 1)],
                                    et[:], start=(c == 0), stop=(c == ncn - 1))
                        for hh, patt in ((0, pA), (1, pB)):
                            inv = sb.tile([1, 512], F32, tag="inv")
                            nc.vector.reciprocal(inv[0:1, :], patt[64:65, :])
                            pbc = ps1.tile([128, 512], F32, tag="pbc")
                            nc.tensor.matmul(pbc[0:64, :], onef[0:1, 0:64],
                                             inv[0:1, :], start=True, stop=True)
                            bcs = sb.tile([64, 512], F32, tag="bcs")
                            nc.vector.tensor_copy(bcs[:], pbc[0:64, :])
                            nc.vector.tensor_tensor(
                                out=attnT[64 * hh:64 * (hh + 1),
                                          1024 * b + 512 * j:
                                          1024 * b + 512 * (j + 1)],
                                in0=patt[0:64, :], in1=bcs[:], op=ALU.mult)

                # wo partial -> bounce
                for m in range(KT):
                    for n in range(NCH):
                        cs = slice(512 * n, 512 * (n + 1))
                        pw = ps2.tile([128, 512], F32, tag="pbig")
                        nc.tensor.matmul(pw[:],
                                         wt[:, (24 + m) * 128:(24 + m + 1) * 128],
                                         attnT[:, cs], start=True, stop=True)
                        st = sb.tile([128, 512], F32, tag="stage")
                        nc.vector.tensor_copy(st[:], pw[:])
                        nc.sync.dma_start(arin[128 * m:128 * (m + 1), cs], st[:])
                allreduce(arin, arout)
                add_from(arout)

                # ================= MoE =================
                rrow = rmsnorm_rstd()
                gwt = per.tile([128, KT * E], F32, tag="gwt")
                nc.sync.dma_start(gwt[:].rearrange("p (a b) -> p a b", a=KT),
                                  gw[l].rearrange("a p b -> p a b"))
                if gb_any:
                    gbt = wp.tile([128, E], F32, tag="gbt")
                    nc.sync.dma_start(gbt[:], gbb[l])
                if egb_any:
                    egbt = wp.tile([128, MT], F32, tag="egbt")
                    nc.sync.dma_start(egbt[:], egb[l])
                if eub_any:
                    eubt = wp.tile([128, MT], F32, tag="eubt")
                    nc.sync.dma_start(eubt[:], eub[l])
                if edb_any:
                    edbt = wp.tile([128, KT], F32, tag="edbt")
                    nc.sync.dma_start(edbt[:], edb[l])

                combT = per.tile([E, T], F32, tag="combT")
                putil = ps1.tile([E, 1], F32, tag="prow")
                for n in range(NCH):
                    cs = slice(512 * n, 512 * (n + 1))
                    pz = ps1.tile([E, 512], F32, tag="pbc")
                    for k in range(KT):
                        nc.tensor.matmul(pz[:], gwt[:, k * E:(k + 1) * E],
                                         xk(k)[:, cs], start=(k == 0),
                                         stop=(k == KT - 1))
                    zs = sb.tile([E, 512], F32, tag="zs")
                    nc.vector.tensor_copy(zs[:], pz[:])
                    for tt_ in range(4):
                        t = 4 * n + tt_
                        # z token-major via PE transpose
                        pzt = ps2.tile([128, E], F32, tag="psml")
                        nc.tensor.transpose(pzt[:],
                                            zs[:, 128 * tt_:128 * (tt_ + 1)],
                                            ident[0:E, 0:E])
                        # rstd token-major
                        prt = ps2.tile([128, 1], F32, tag="psml")
                        nc.tensor.matmul(prt[:],
                                         rrow[0:1, 128 * t:128 * (t + 1)],
                                         onef[0:1, 0:1], start=True, stop=True)
                        rts = sb.tile([128, 1], F32, tag="rts")
                        nc.vector.tensor_copy(rts[:], prt[:])
                        lg = sb.tile([128, 8], F32, tag="lg")
                        nc.vector.memset(lg[:, 4:8], -3e38)
                        nc.vector.tensor_scalar(out=lg[:, 0:E], in0=pzt[:],
                                                scalar1=rts[:, 0:1], scalar2=None,
                                                op0=ALU.mult)
                        if gb_any:
                            nc.vector.tensor_add(lg[:, 0:E], lg[:, 0:E], gbt[:])
                        top8 = sb.tile([128, 8], F32, tag="top8")
                        nc.vector.max(out=top8[:], in_=lg[:])
                        ngm = sb.tile([128, 1], F32, tag="ngm")
                        nc.vector.tensor_scalar_mul(ngm[:], top8[:, 0:1], -1.0)
                        ex = sb.tile([128, E], F32, tag="exe")
                        nc.scalar.activation(ex[:], lg[:, 0:E], AF.Exp,
                                             bias=ngm[:, 0:1])
                        msk = sb.tile([128, E], F32, tag="msk")
                        nc.vector.tensor_scalar(out=msk[:], in0=lg[:, 0:E],
                                                scalar1=top8[:, 1:2],
                                                scalar2=None, op0=ALU.is_ge)
                        nc.tensor.matmul(putil[:], msk[:], onef[:, 0:1],
                                         start=(t == 0), stop=(t == TT - 1))
                        num = sb.tile([128, E], F32, tag="num")
                        nc.vector.tensor_tensor(out=num[:], in0=ex[:],
                                                in1=msk[:], op=ALU.mult)
                        den = sb.tile([128, 1], F32, tag="den")
                        nc.vector.reduce_sum(den[:], num[:], axis=AX.X)
                        inv2 = sb.tile([128, 1], F32, tag="inv2")
                        nc.vector.reciprocal(inv2[:], den[:])
                        cmb = sb.tile([128, E], F32, tag="cmb")
                        nc.vector.tensor_scalar_mul(cmb[:], num[:],
                                                    inv2[:, 0:1])
                        pct = ps2.tile([E, 128], F32, tag="psml")
                        nc.tensor.transpose(pct[:], cmb[:], ident[:])
                        nc.vector.tensor_copy(combT[:, 128 * t:128 * (t + 1)],
                                              pct[:])
                # balance loss contribution
                us = sb.tile([E, 1], F32, tag="us")
                nc.vector.tensor_scalar(out=us[:], in0=putil[:],
                                        scalar1=1.0 / T, scalar2=-1.0 / E,
                                        op0=ALU.mult, op1=ALU.add)
                nc.scalar.activation(us[:], us[:], AF.Square)
                pbal = ps1.tile([1, 1], F32, tag="prow")
                nc.tensor.matmul(pbal[:], us[:], onef[0:E, 0:1],
                                 start=True, stop=True)
                nc.vector.tensor_add(balsb[:], balsb[:], pbal[:])

                # expert FF (this core's expert/FF-half), dense over tokens
                for n in range(NCH):
                    cs = slice(512 * n, 512 * (n + 1))
                    h2c = norm_chunk(rrow, n, "h2c")
                    # combine-weight broadcast for this core's expert
                    pcb = ps1.tile([128, 512], F32, tag="pbc")
                    nc.tensor.matmul(pcb[:], eselv[:], combT[:, cs],
                                     start=True, stop=True)
                    cbs = per.tile([128, 512], F32, tag="cbs")
                    nc.vector.tensor_copy(cbs[:], pcb[:])
                    prod = per.tile([128, MT * 512], BF16, tag="prod")
                    for m in range(MT):
                        egt = wp.tile([128, KT * 128], BF16, tag="egt")
                        nc.sync.dma_start(
                            egt[:].rearrange("p (a b) -> p a b", a=KT),
                            egw[l, m].rearrange("a p b -> p a b"))
                        eut = wp.tile([128, KT * 128], BF16, tag="eut")
                        nc.sync.dma_start(
                            eut[:].rearrange("p (a b) -> p a b", a=KT),
                            euw[l, m].rearrange("a p b -> p a b"))
                        pg = ps2.tile([128, 512], F32, tag="pbig")
                        for k in range(KT):
                            nc.tensor.matmul(pg[:],
                                             egt[:, 128 * k:128 * (k + 1)],
                                             h2c[:, 512 * k:512 * (k + 1)],
                                             start=(k == 0), stop=(k == KT - 1))
                        gg = sb.tile([128, 512], BF16, tag="gg")
                        nc.scalar.activation(
                            gg[:], pg[:], AF.Gelu,
                            bias=(egbt[:, m:m + 1] if egb_any else 0.0))
                        pu = ps2.tile([128, 512], F32, tag="pbig")
                        for k in range(KT):
                            nc.tensor.matmul(pu[:],
                                             eut[:, 128 * k:128 * (k + 1)],
                                             h2c[:, 512 * k:512 * (k + 1)],
                                             start=(k == 0), stop=(k == KT - 1))
                        if eub_any:
                            nc.vector.tensor_scalar(
                                out=pu[:], in0=pu[:],
                                scalar1=eubt[:, m:m + 1], scalar2=None,
                                op0=ALU.add)
                        nc.vector.tensor_tensor(
                            out=prod[:, 512 * m:512 * (m + 1)],
                            in0=pu[:], in1=gg[:], op=ALU.mult)
                    for mh in range(KT):
                        edt = wp.tile([128, MT * 128], BF16, tag="edt")
                        nc.sync.dma_start(
                            edt[:].rearrange("p (a b) -> p a b", a=MT),
                            edw[l, mh].rearrange("a p b -> p a b"))
                        pe = ps2.tile([128, 512], F32, tag="pbig")
                        for kf in range(MT):
                            nc.tensor.matmul(pe[:],
                                             edt[:, 128 * kf:128 * (kf + 1)],
                                             prod[:, 512 * kf:512 * (kf + 1)],
                                             start=(kf == 0), stop=(kf == MT - 1))
                        if edb_any:
                            nc.vector.tensor_scalar(
                                out=pe[:], in0=pe[:],
                                scalar1=edbt[:, mh:mh + 1], scalar2=None,
                                op0=ALU.add)
                        st = sb.tile([128, 512], F32, tag="stage")
                        nc.vector.tensor_tensor(out=st[:], in0=pe[:],
                                                in1=cbs[:], op=ALU.mult)
                        nc.sync.dma_start(arin2[128 * mh:128 * (mh + 1), cs],
                                          st[:])
                allreduce(arin2, arout2)
                add_from(arout2)

            # ================= final norm + lm_head =================
            nc.vector.tensor_copy(balsb[:], balsb[:])
            nc.sync.dma_start(bal_o[:], balsb[:])
            rrow = rmsnorm_rstd()
            for n in range(NCH):
                hfc = norm_chunk(rrow, n, "hc")
                for tt_ in range(4):
                    t = 4 * n + tt_
                    for vch in range(8):
                        vs = slice(500 * vch, 500 * (vch + 1))
                        pl = ps2.tile([128, 500], F32, tag="pbig")
                        for k in range(KT):
                            wl = wp.tile([128, 500], BF16, tag="wl")
                            nc.sync.dma_start(wl[:], wlm[k][:, vs])
                            nc.tensor.matmul(
                                pl[:],
                                hfc[:, 512 * k + 128 * tt_:
                                    512 * k + 128 * (tt_ + 1)],
                                wl[:], start=(k == 0), stop=(k == KT - 1))
                        st = sb.tile([128, 500], F32, tag="stage")
                        nc.vector.tensor_copy(st[:], pl[:])
                        nc.sync.dma_start(
                            logits[128 * t:128 * (t + 1), vs], st[:])

    nc.compile()
    return nc


_cache = {}


def kernel(input_ids, token_emb, norm1_w, wq, wk, wv, wo, norm2_w,
           gate_w, gate_b, eg_w, eg_b, eu_w, eu_b, ed_w, ed_b,
           normf_w, lm_head, _trace=False):
    global LAST_EXEC_NS
    ids = np.asarray(input_ids).reshape(-1)
    emb = f32(token_emb)

    x0T = f32((emb[ids] * np.float32(np.sqrt(float(H)))).T)

    theta = 1.0 / (ROPE_BASE ** (np.arange(0, HD, 2, dtype=np.float64) / HD))
    ang = np.outer(np.arange(S, dtype=np.float64), theta)
    cosv, sinv = f32(np.cos(ang).T), f32(np.sin(ang).T)

    m4 = np.zeros((128, 4, 512), np.float32)
    p = np.arange(128)[:, None]
    tq = np.arange(512)[None, :]
    for jj in range(4):
        m4[:, jj, :] = np.where(128 * jj + p <= tq, 0.0, MASKV)
    m4 = f32(m4.reshape(128, 4 * 512))

    gb_any = bool(np.any(gate_b))
    egb_any = bool(np.any(eg_b))
    eub_any = bool(np.any(eu_b))
    edb_any = bool(np.any(ed_b))
    wb = (gb_any, egb_any, eub_any, edb_any)
    if wb not in _cache:
        _cache[wb] = build(wb)
    nc = _cache[wb]

    n1 = f32(norm1_w)[:, :, None]     # [L, H, 1]
    n2 = f32(norm2_w)[:, :, None]
    in_maps = []
    for c in range(NC):
        hs = slice(128 * c, 128 * (c + 1))
        fs = slice(FFL * (c % 2), FFL * (c % 2 + 1))
        e = c // 2
        wq_s = bf((f32(wq)[:, :, hs] * n1).reshape(L, KT, 128, 128))
        wk_s = bf((f32(wk)[:, :, hs] * n1).reshape(L, KT, 128, 128))
        wv_s = bf((f32(wv)[:, :, hs] * n1).reshape(L, KT, 128, 128))
        wo_s = bf(f32(wo)[:, hs, :].reshape(L, 128, KT, 128)
                  .transpose(0, 2, 1, 3))
        wqkvo = np.concatenate([wq_s, wk_s, wv_s, wo_s], axis=1)
        gw_s = f32((f32(gate_w) * n2).reshape(L, KT, 128, E))
        egw_s = bf((f32(eg_w)[:, e, :, fs] * n2).reshape(L, KT, 128, MT, 128)
                   .transpose(0, 3, 1, 2, 4))
        euw_s = bf((f32(eu_w)[:, e, :, fs] * n2).reshape(L, KT, 128, MT, 128)
                   .transpose(0, 3, 1, 2, 4))
        edw_s = bf(f32(ed_w)[:, e, fs, :].reshape(L, MT, 128, KT, 128)
                   .transpose(0, 3, 1, 2, 4))
        vs = slice(VL * c, VL * (c + 1))
        wlm_s = bf((f32(lm_head)[:, vs] * f32(normf_w)[:, None])
                   .reshape(KT, 128, VL))
        ese = np.zeros((E, 128), np.float32)
        ese[e, :] = 1.0
        m = dict(x0T=x0T, wqkvo=wqkvo, gw=gw_s, egw=egw_s, euw=euw_s,
                 edw=edw_s, wlm=wlm_s, cosT=cosv, sinT=sinv, mask4=m4,
                 esel=ese)
        if gb_any:
            m["gbb"] = f32(np.tile(f32(gate_b)[:, None, :], (1, 128, 1)))
        if egb_any:
            m["egb"] = f32(f32(eg_b)[:, e, fs].reshape(L, MT, 128)
                           .transpose(0, 2, 1))
        if eub_any:
            m["eub"] = f32(f32(eu_b)[:, e, fs].reshape(L, MT, 128)
                           .transpose(0, 2, 1))
        if edb_any:
            m["edb"] = f32(f32(ed_b)[:, e, :].reshape(L, KT, 128)
                           .transpose(0, 2, 1))
        in_maps.append(m)

    if _trace:
        _install_ntff_hook()
    res = bass_utils.run_bass_kernel_spmd(
        nc, in_maps, core_ids=list(range(NC)), trace=_trace)
    LAST_EXEC_NS = res.exec_time_ns

    lg = np.concatenate([np.asarray(res.results[c]["logits"], np.float32)
                         for c in range(NC)], axis=1)
    bal = np.float32(np.asarray(res.results[0]["bal"], np.float32)[0, 0])
    return lg.reshape(B, S, V), bal
